# revision 10
# baseline (speedup 1.0000x reference)
"""GCN layer on 8 trn2 NeuronCores (Bass/Tile via PJRT).

out = relu( (D^-1/2 A D^-1/2) x W^T + b )

Strategy:
  - dst nodes sharded 8 ways (12500/core); each core processes exactly the
    edges whose destination lands in its shard (host bucketing, int-only).
  - z = dis * (x @ W^T) computed on-device per core (replicated), stored to
    device DRAM. Linear layer commutes with the (linear) aggregation, so the
    64x64 matmul runs on the small side of the gather.
  - per-edge gather of z rows via vector-indirect DMA ([128,1] offsets);
    K-slot accumulation on DVE: destination nodes are degree-sorted so round
    s covers a prefix of nodes, eliminating slot padding.
  - finalize: scale by dis_dst, apply W^T via PE (needs a transpose), add b,
    relu, write the core's output shard.

Host does integer index work only (bucketing, sorting, bincount of indices,
layout); all floating-point math runs on device.
"""
import time
from contextlib import ExitStack

import numpy as np

N_NODES = 100000
N_EDGES = 1600000
D = 64
NC = 8
SH = N_NODES // NC              # 12500 dst nodes per core
PADN = 12800                    # core dst rows padded (100 blocks of 128)
NBLK = PADN // 128              # 100
NPADZ = 100352                  # z rows padded (784 * 128)
ZERO_IDX = 100096               # a guaranteed-zero z row (beyond real nodes)
ZTILES = NPADZ // 128           # 784
GROUP = 512                     # round widths padded to 512 (4 blocks of 128)

_CACHE = {}


# ---------------------------------------------------------------- host prep

def _host_prep(row, col):
    """Integer-only preprocessing. Returns global round structure and
    per-core arrays."""
    deg = np.bincount(row, minlength=N_NODES).astype(np.int64)

    per_core = []
    for k in range(NC):
        lo, hi = k * SH, (k + 1) * SH
        m = (row >= lo) & (row < hi)
        r = row[m] - lo
        c = col[m]
        dk = deg[lo:hi]
        perm = np.argsort(-dk, kind="stable")          # sorted pos -> orig local
        pos = np.empty(SH, np.int64)
        pos[perm] = np.arange(SH)
        o = np.argsort(pos[r], kind="stable")
        rpos = pos[r][o]
        csrt = c[o]
        starts = np.searchsorted(rpos, np.arange(SH + 1))
        dsort = dk[perm]                               # degrees, descending
        per_core.append((perm, csrt, starts, dsort))

    kmax = int(max(pc[3][0] if len(pc[3]) else 0 for pc in per_core))
    # global round widths (padded to GROUP)
    widths = []
    for s in range(kmax):
        cnt = max(int(np.searchsorted(-pc[3], -s, side="left"))
                  for pc in per_core)
        w = min(-(-cnt // GROUP) * GROUP, PADN)
        widths.append(w)
    ncols = sum(w // 128 for w in widths)

    g_arrs, degloc_arrs, perms = [], [], []
    for k in range(NC):
        perm, csrt, starts, dsort = per_core[k]
        G = np.full((128, ncols), ZERO_IDX, np.int32)
        colptr = 0
        for s in range(kmax):
            w = widths[s]
            cnt = int(np.searchsorted(-dsort, -s, side="left"))
            j = np.arange(min(cnt, w))
            vals = csrt[starts[j] + s]
            G[j % 128, colptr + j // 128] = vals.astype(np.int32)
            colptr += w // 128
        g_arrs.append(G)
        dloc = np.zeros(PADN, np.int64)
        dloc[:SH] = dsort
        degloc_arrs.append(dloc.reshape(NBLK, 128).T.astype(np.int32).copy())
        perms.append(perm)

    deg_pad = np.zeros(NPADZ, np.int64)
    deg_pad[:N_NODES] = deg
    deg_pn = deg_pad.reshape(ZTILES, 128).T.astype(np.int32).copy()

    return widths, ncols, g_arrs, degloc_arrs, perms, deg_pn


# ---------------------------------------------------------------- program

def _build_program(widths, ncols):
    import concourse.bacc as bacc
    import concourse.mybir as mybir
    import concourse.tile as tile
    from concourse.masks import make_identity

    f32 = mybir.dt.float32
    i32 = mybir.dt.int32

    nc = bacc.Bacc("TRN2", target_bir_lowering=False)
    g_xT = nc.declare_dram_parameter("g_xT", [D, NPADZ], f32, isOutput=False)
    g_wt = nc.declare_dram_parameter("g_wt", [D, D], f32, isOutput=False)
    g_b = nc.declare_dram_parameter("g_b", [1, D], f32, isOutput=False)
    g_degpn = nc.declare_dram_parameter("g_degpn", [128, ZTILES], i32, isOutput=False)
    g_degloc = nc.declare_dram_parameter("g_degloc", [128, NBLK], i32, isOutput=False)
    g_gidx = nc.declare_dram_parameter("g_gidx", [128, ncols], i32, isOutput=False)
    g_out = nc.declare_dram_parameter("g_out", [PADN, D], f32, isOutput=True)
    z_dram = nc.dram_tensor("z_dram", [NPADZ, D], f32)

    import concourse.bass as bass

    # persistent SBUF state shared across the sequential TileContexts
    acc_t = nc.alloc_sbuf_tensor("acc_sb", [128, NBLK, D], f32)
    gidx_t = nc.alloc_sbuf_tensor("gidx_sb", [128, ncols], i32)
    disloc_t = nc.alloc_sbuf_tensor("disloc_sb", [128, NBLK], f32)
    bfull_t = nc.alloc_sbuf_tensor("bfull_sb", [128, D], f32)
    acc = acc_t.ap()
    gidx_sb = gidx_t.ap()
    dis_loc = disloc_t.ap()
    b_full = bfull_t.ap()

    # split gather columns into segments (sem values reset per TileContext)
    GSEG = 640
    seg_bounds = []
    cstart = 0
    cur = 0
    segs = []
    for si, w in enumerate(widths):
        segs.append(w)
        cur += w // 128
        if cur - cstart >= GSEG or si == len(widths) - 1:
            seg_bounds.append((cstart, cur, segs))
            cstart = cur
            segs = []

    # ---------- context 1: constants, dis, z phase ----------
    with tile.TileContext(nc) as tc, ExitStack() as ctx:
        cp = ctx.enter_context(tc.tile_pool(name="cp", bufs=1))
        xp = ctx.enter_context(tc.tile_pool(name="xp", bufs=3))
        zp = ctx.enter_context(tc.tile_pool(name="zp", bufs=3))
        pp = ctx.enter_context(tc.tile_pool(name="pp", bufs=2, space="PSUM"))

        wt_sb = cp.tile([D, D], f32)
        nc.sync.dma_start(out=wt_sb[:], in_=g_wt[:])
        b_sb = cp.tile([1, D], f32)
        nc.sync.dma_start(out=b_sb[:], in_=g_b[:])
        nc.sync.dma_start(out=gidx_sb[:], in_=g_gidx[:])
        ones_sb = cp.tile([1, 128], f32)
        nc.vector.memset(ones_sb[:], 1.0)

        psum_b = pp.tile([128, D], f32, tag="py")
        nc.tensor.matmul(psum_b[:], lhsT=ones_sb[:], rhs=b_sb[:],
                         start=True, stop=True)
        nc.vector.tensor_copy(out=b_full[:], in_=psum_b[:])
        nc.vector.memset(acc[:], 0.0)

        def make_dis(deg_dram, width, pool, tg, dis_out):
            di = pool.tile([128, width], i32, tag=f"di_{tg}")
            nc.sync.dma_start(out=di[:], in_=deg_dram[:])
            df = pool.tile([128, width], f32, tag=f"df_{tg}")
            nc.vector.tensor_copy(out=df[:], in_=di[:])
            t = pool.tile([128, width], f32, tag=f"t_{tg}")
            nc.vector.tensor_scalar_max(out=t[:], in0=df[:], scalar1=1.0)
            s = pool.tile([128, width], f32, tag=f"s_{tg}")
            nc.scalar.activation(s[:], t[:], mybir.ActivationFunctionType.Sqrt)
            r = pool.tile([128, width], f32, tag=f"r_{tg}")
            nc.vector.reciprocal(out=r[:], in_=s[:])
            mk = pool.tile([128, width], f32, tag=f"mk_{tg}")
            nc.vector.tensor_scalar(out=mk[:], in0=df[:], scalar1=0.0,
                                    scalar2=None, op0=mybir.AluOpType.is_gt)
            nc.vector.tensor_tensor(out=dis_out[:], in0=r[:], in1=mk[:],
                                    op=mybir.AluOpType.mult)

        disp = cp.tile([128, ZTILES], f32, tag="dis_pn")
        make_dis(g_degpn, ZTILES, cp, "pn", disp)
        make_dis(g_degloc, NBLK, cp, "loc", dis_loc)

        for jc in range(ZTILES // 8):
            xc = xp.tile([D, 1024], f32)
            nc.sync.dma_start(out=xc[:], in_=g_xT[:, jc * 1024:(jc + 1) * 1024])
            zstage = zp.tile([128, 8, D], f32)
            for t8 in range(8):
                j = jc * 8 + t8
                py = pp.tile([128, D], f32, tag="py")
                nc.tensor.matmul(py[:], lhsT=xc[:, t8 * 128:(t8 + 1) * 128],
                                 rhs=wt_sb[:], start=True, stop=True)
                nc.scalar.activation(zstage[:, t8, :], py[:],
                                     mybir.ActivationFunctionType.Copy,
                                     scale=disp[:, j:j + 1])
            nc.sync.dma_start(
                out=z_dram[jc * 1024:(jc + 1) * 1024, :].rearrange(
                    "(a p) d -> p a d", p=128),
                in_=zstage[:],
            )

    # ---------- contexts 2..n: gather/accumulate segments ----------
    for cs, ce, wlist in seg_bounds:
        with tile.TileContext(nc) as tc, ExitStack() as ctx:
            gp = ctx.enter_context(tc.tile_pool(name="gp", bufs=6))
            cidx = cs
            for w in wlist:
                nb = w // 128
                for gblk in range(nb // 4):
                    stage = gp.tile([128, 4, D], f32)
                    for q in range(4):
                        nc.gpsimd.indirect_dma_start(
                            out=stage[:, q, :],
                            out_offset=None,
                            in_=z_dram[:],
                            in_offset=bass.IndirectOffsetOnAxis(
                                ap=gidx_sb[:, cidx:cidx + 1], axis=0),
                        )
                        cidx += 1
                    blk = gblk * 4
                    nc.vector.tensor_tensor(
                        out=acc[:, blk:blk + 4, :], in0=acc[:, blk:blk + 4, :],
                        in1=stage[:], op=mybir.AluOpType.add)
            assert cidx == ce

    # ---------- final context: scale by dis_dst, +b, relu, store ----------
    with tile.TileContext(nc) as tc, ExitStack() as ctx:
        op = ctx.enter_context(tc.tile_pool(name="op", bufs=3))
        for j in range(NBLK):
            t1 = op.tile([128, D], f32)
            nc.scalar.activation(t1[:], acc[:, j, :],
                                 mybir.ActivationFunctionType.Copy,
                                 scale=dis_loc[:, j:j + 1])
            t2 = op.tile([128, D], f32)
            nc.vector.tensor_tensor(out=t2[:], in0=t1[:], in1=b_full[:],
                                    op=mybir.AluOpType.add)
            t3 = op.tile([128, D], f32)
            nc.vector.tensor_scalar_max(out=t3[:], in0=t2[:], scalar1=0.0)
            nc.sync.dma_start(out=g_out[j * 128:(j + 1) * 128, :], in_=t3[:])

    nc.compile()
    return nc


# ---------------------------------------------------------------- runner

def _build_runner(nc, n_cores=NC):
    import jax
    from jax.sharding import Mesh, PartitionSpec
    from jax.experimental.shard_map import shard_map
    import concourse.mybir as mybir
    from concourse import bass2jax

    bass2jax.install_neuronx_cc_hook()
    partition_name = nc.partition_id_tensor.name if nc.partition_id_tensor else None

    in_names, out_names, out_avals, zero_outs = [], [], [], []
    for alloc in nc.m.functions[0].allocations:
        if not isinstance(alloc, mybir.MemoryLocationSet):
            continue
        name = alloc.memorylocations[0].name
        if alloc.kind == "ExternalInput":
            if name != partition_name:
                in_names.append(name)
        elif alloc.kind == "ExternalOutput":
            out_names.append(name)
            shape = tuple(alloc.tensor_shape)
            dtype = mybir.dt.np(alloc.dtype)
            out_avals.append(jax.core.ShapedArray(shape, dtype))
            zero_outs.append(np.zeros(shape, dtype))
    n_params = len(in_names)
    n_outs = len(out_avals)
    all_in_names = list(in_names) + list(out_names)
    if partition_name is not None:
        all_in_names.append(partition_name)

    def _body(*args):
        operands = list(args)
        if partition_name is not None:
            operands.append(bass2jax.partition_id_tensor())
        outs = bass2jax._bass_exec_p.bind(
            *operands,
            out_avals=tuple(out_avals),
            in_names=tuple(all_in_names),
            out_names=tuple(out_names),
            lowering_input_output_aliases=(),
            sim_require_finite=True,
            sim_require_nnan=True,
            nc=nc,
        )
        return tuple(outs)

    devices = jax.devices()[:n_cores]
    mesh = Mesh(np.asarray(devices), ("core",))
    in_specs = (PartitionSpec("core"),) * (n_params + n_outs)
    out_specs = (PartitionSpec("core"),) * n_outs
    donate = tuple(range(n_params, n_params + n_outs))
    fn = jax.jit(
        shard_map(_body, mesh=mesh, in_specs=in_specs, out_specs=out_specs,
                  check_rep=False),
        donate_argnums=donate,
        keep_unused=True,
    )
    sharding = jax.sharding.NamedSharding(mesh, PartitionSpec("core"))

    class R:
        pass

    r = R()
    r.fn = fn
    r.in_names = in_names
    r.out_names = out_names
    r.zero_outs = zero_outs
    r.sharding = sharding
    r.n_cores = n_cores

    def prep_inputs(in_maps):
        import jax as _jax
        return [
            _jax.device_put(
                np.concatenate([np.asarray(m[nm]) for m in in_maps], axis=0),
                sharding)
            for nm in in_names
        ]

    def put_zero_outs():
        import jax as _jax
        return [_jax.device_put(np.concatenate([z] * n_cores, axis=0), sharding)
                for z in zero_outs]

    def run(dev_inputs):
        import jax as _jax
        outs = fn(*dev_inputs, *put_zero_outs())
        outs = [np.asarray(o) for o in outs]
        res = []
        for c in range(n_cores):
            d = {}
            for name, glob in zip(out_names, outs):
                per = glob.shape[0] // n_cores
                d[name] = glob[c * per:(c + 1) * per]
            res.append(d)
        return res

    r.prep_inputs = prep_inputs
    r.put_zero_outs = put_zero_outs
    r.run = run
    return r


# ---------------------------------------------------------------- kernel

def kernel(x, edge_index, W, b):
    x = np.ascontiguousarray(np.asarray(x, dtype=np.float32))
    ei = np.asarray(edge_index)
    W = np.asarray(W, dtype=np.float32)
    b = np.asarray(b, dtype=np.float32)
    row = ei[0].astype(np.int64)
    col = ei[1].astype(np.int64)

    widths, ncols, g_arrs, degloc_arrs, perms, deg_pn = _host_prep(row, col)

    key = (tuple(widths), ncols)
    if key not in _CACHE:
        nc = _build_program(widths, ncols)
        _CACHE[key] = (nc, _build_runner(nc))
    nc, runner = _CACHE[key]

    xT = np.zeros((D, NPADZ), np.float32)
    xT[:, :N_NODES] = x.T
    wt = np.ascontiguousarray(W.T)          # [in, out]
    b2 = b.reshape(1, D)

    in_maps = []
    for k in range(NC):
        in_maps.append({
            "g_xT": xT,
            "g_wt": wt,
            "g_b": b2,
            "g_degpn": deg_pn,
            "g_degloc": degloc_arrs[k],
            "g_gidx": g_arrs[k],
        })
    dev_in = runner.prep_inputs(in_maps)
    res = runner.run(dev_in)

    out = np.empty((N_NODES, D), np.float32)
    for k in range(NC):
        ok = res[k]["g_out"][:SH]
        blk = np.empty((SH, D), np.float32)
        blk[perms[k]] = ok
        out[k * SH:(k + 1) * SH] = blk
    return out


# revision 13
# speedup vs baseline: 25.7336x; 25.7336x over previous
"""GCN layer on 8 trn2 NeuronCores (Bass/Tile via PJRT).

out = relu( (D^-1/2 A D^-1/2) x W^T + b )

Strategy:
  - dst nodes sharded 8 ways (12500/core); each core processes exactly the
    edges whose destination lands in its shard (host bucketing, int-only).
  - z = dis * (x @ W^T) computed on-device per core (replicated), stored to
    device DRAM. Linear layer commutes with the (linear) aggregation, so the
    64x64 matmul runs on the small side of the gather.
  - per-edge gather of z rows via vector-indirect DMA ([128,1] offsets);
    K-slot accumulation on DVE: destination nodes are degree-sorted so round
    s covers a prefix of nodes, eliminating slot padding.
  - finalize: scale by dis_dst, apply W^T via PE (needs a transpose), add b,
    relu, write the core's output shard.

Host does integer index work only (bucketing, sorting, bincount of indices,
layout); all floating-point math runs on device.
"""
import time
from contextlib import ExitStack

import numpy as np

N_NODES = 100000
N_EDGES = 1600000
D = 64
NC = 8
SH = N_NODES // NC              # 12500 dst nodes per core
PADN = 12800                    # core dst rows padded (100 blocks of 128)
NBLK = PADN // 128              # 100
NPADZ = 100352                  # z rows padded (784 * 128)
ZERO_IDX = 100096               # a guaranteed-zero z row (beyond real nodes)
ZTILES = NPADZ // 128           # 784
GROUP = 512                     # round widths padded to 512 (4 blocks of 128)

_CACHE = {}


# ---------------------------------------------------------------- host prep

def _host_prep(row, col):
    """Integer-only preprocessing. Returns global round structure and
    per-core arrays."""
    deg = np.bincount(row, minlength=N_NODES).astype(np.int64)

    per_core = []
    for k in range(NC):
        lo, hi = k * SH, (k + 1) * SH
        m = (row >= lo) & (row < hi)
        r = row[m] - lo
        c = col[m]
        dk = deg[lo:hi]
        perm = np.argsort(-dk, kind="stable")          # sorted pos -> orig local
        pos = np.empty(SH, np.int64)
        pos[perm] = np.arange(SH)
        o = np.argsort(pos[r], kind="stable")
        rpos = pos[r][o]
        csrt = c[o]
        starts = np.searchsorted(rpos, np.arange(SH + 1))
        dsort = dk[perm]                               # degrees, descending
        per_core.append((perm, csrt, starts, dsort))

    kmax = int(max(pc[3][0] if len(pc[3]) else 0 for pc in per_core))
    # global round widths (padded to GROUP)
    widths = []
    for s in range(kmax):
        cnt = max(int(np.searchsorted(-pc[3], -s, side="left"))
                  for pc in per_core)
        w = min(-(-cnt // GROUP) * GROUP, PADN)
        widths.append(w)
    ncols = sum(w // 128 for w in widths)

    g_arrs, degloc_arrs, perms = [], [], []
    for k in range(NC):
        perm, csrt, starts, dsort = per_core[k]
        G = np.full((128, ncols), ZERO_IDX, np.int32)
        colptr = 0
        for s in range(kmax):
            w = widths[s]
            cnt = int(np.searchsorted(-dsort, -s, side="left"))
            j = np.arange(min(cnt, w))
            vals = csrt[starts[j] + s]
            G[j % 128, colptr + j // 128] = vals.astype(np.int32)
            colptr += w // 128
        g_arrs.append(G)
        dloc = np.zeros(PADN, np.int64)
        dloc[:SH] = dsort
        degloc_arrs.append(dloc.reshape(NBLK, 128).T.astype(np.int32).copy())
        perms.append(perm)

    deg_pad = np.zeros(NPADZ, np.int64)
    deg_pad[:N_NODES] = deg
    deg_pn = deg_pad.reshape(ZTILES, 128).T.astype(np.int32).copy()

    return widths, ncols, g_arrs, degloc_arrs, perms, deg_pn


# ---------------------------------------------------------------- program

def _build_program(widths, ncols, repeat=1):
    import concourse.bacc as bacc
    import concourse.mybir as mybir
    import concourse.tile as tile
    from concourse.masks import make_identity

    f32 = mybir.dt.float32
    i32 = mybir.dt.int32

    nc = bacc.Bacc("TRN2", target_bir_lowering=False)
    g_xT = nc.declare_dram_parameter("g_xT", [D, NPADZ], f32, isOutput=False)
    g_wt = nc.declare_dram_parameter("g_wt", [D, D], f32, isOutput=False)
    g_b = nc.declare_dram_parameter("g_b", [1, D], f32, isOutput=False)
    g_degpn = nc.declare_dram_parameter("g_degpn", [128, ZTILES], i32, isOutput=False)
    g_degloc = nc.declare_dram_parameter("g_degloc", [128, NBLK], i32, isOutput=False)
    g_gidx = nc.declare_dram_parameter("g_gidx", [128, ncols], i32, isOutput=False)
    g_out = nc.declare_dram_parameter("g_out", [PADN, D], f32, isOutput=True)
    z_dram = nc.dram_tensor("z_dram", [NPADZ, D], f32)

    import concourse.bass as bass

    # persistent SBUF state shared across the sequential TileContexts
    acc_t = nc.alloc_sbuf_tensor("acc_sb", [128, NBLK, D], f32)
    gidx_t = nc.alloc_sbuf_tensor("gidx_sb", [128, ncols], i32)
    disloc_t = nc.alloc_sbuf_tensor("disloc_sb", [128, NBLK], f32)
    bfull_t = nc.alloc_sbuf_tensor("bfull_sb", [128, D], f32)
    acc = acc_t.ap()
    gidx_sb = gidx_t.ap()
    dis_loc = disloc_t.ap()
    b_full = bfull_t.ap()

    # split gather columns into segments (sem values reset per TileContext)
    GSEG = 640
    seg_bounds = []
    cstart = 0
    cur = 0
    segs = []
    for si, w in enumerate(widths):
        segs.append(w)
        cur += w // 128
        if cur - cstart >= GSEG or si == len(widths) - 1:
            seg_bounds.append((cstart, cur, segs))
            cstart = cur
            segs = []

    for _rep in range(repeat):
        _build_body(nc, tile, mybir, bass, make_identity, f32, i32,
                    widths, ncols, seg_bounds,
                    g_xT, g_wt, g_b, g_degpn, g_degloc, g_gidx, g_out, z_dram,
                    acc, gidx_sb, dis_loc, b_full)

    nc.compile()
    return nc


def _build_body(nc, tile, mybir, bass, make_identity, f32, i32,
                widths, ncols, seg_bounds,
                g_xT, g_wt, g_b, g_degpn, g_degloc, g_gidx, g_out, z_dram,
                acc, gidx_sb, dis_loc, b_full):
    D_ = D
    ZT = ZTILES

    # ---------- context 1: constants, dis, z phase ----------
    with tile.TileContext(nc) as tc, ExitStack() as ctx:
        cp = ctx.enter_context(tc.tile_pool(name="cp", bufs=1))
        xp = ctx.enter_context(tc.tile_pool(name="xp", bufs=3))
        zp = ctx.enter_context(tc.tile_pool(name="zp", bufs=3))
        pp = ctx.enter_context(tc.tile_pool(name="pp", bufs=2, space="PSUM"))

        wt_sb = cp.tile([D, D], f32)
        nc.sync.dma_start(out=wt_sb[:], in_=g_wt[:])
        b_sb = cp.tile([1, D], f32)
        nc.sync.dma_start(out=b_sb[:], in_=g_b[:])
        nc.sync.dma_start(out=gidx_sb[:], in_=g_gidx[:])
        ones_sb = cp.tile([1, 128], f32)
        nc.vector.memset(ones_sb[:], 1.0)

        psum_b = pp.tile([128, D], f32, tag="py")
        nc.tensor.matmul(psum_b[:], lhsT=ones_sb[:], rhs=b_sb[:],
                         start=True, stop=True)
        nc.vector.tensor_copy(out=b_full[:], in_=psum_b[:])
        nc.vector.memset(acc[:], 0.0)

        def make_dis(deg_dram, width, pool, tg, dis_out):
            di = pool.tile([128, width], i32, tag=f"di_{tg}")
            nc.sync.dma_start(out=di[:], in_=deg_dram[:])
            df = pool.tile([128, width], f32, tag=f"df_{tg}")
            nc.vector.tensor_copy(out=df[:], in_=di[:])
            t = pool.tile([128, width], f32, tag=f"t_{tg}")
            nc.vector.tensor_scalar_max(out=t[:], in0=df[:], scalar1=1.0)
            s = pool.tile([128, width], f32, tag=f"s_{tg}")
            nc.scalar.activation(s[:], t[:], mybir.ActivationFunctionType.Sqrt)
            r = pool.tile([128, width], f32, tag=f"r_{tg}")
            nc.vector.reciprocal(out=r[:], in_=s[:])
            mk = pool.tile([128, width], f32, tag=f"mk_{tg}")
            nc.vector.tensor_scalar(out=mk[:], in0=df[:], scalar1=0.0,
                                    scalar2=None, op0=mybir.AluOpType.is_gt)
            nc.vector.tensor_tensor(out=dis_out[:], in0=r[:], in1=mk[:],
                                    op=mybir.AluOpType.mult)

        disp = cp.tile([128, ZTILES], f32, tag="dis_pn")
        make_dis(g_degpn, ZTILES, cp, "pn", disp)
        make_dis(g_degloc, NBLK, cp, "loc", dis_loc)

        for jc in range(ZTILES // 8):
            xc = xp.tile([D, 1024], f32)
            nc.sync.dma_start(out=xc[:], in_=g_xT[:, jc * 1024:(jc + 1) * 1024])
            zstage = zp.tile([128, 8, D], f32)
            for t8 in range(8):
                j = jc * 8 + t8
                py = pp.tile([128, D], f32, tag="py")
                nc.tensor.matmul(py[:], lhsT=xc[:, t8 * 128:(t8 + 1) * 128],
                                 rhs=wt_sb[:], start=True, stop=True)
                nc.scalar.activation(zstage[:, t8, :], py[:],
                                     mybir.ActivationFunctionType.Copy,
                                     scale=disp[:, j:j + 1])
            nc.sync.dma_start(
                out=z_dram[jc * 1024:(jc + 1) * 1024, :].rearrange(
                    "(a p) d -> p a d", p=128),
                in_=zstage[:],
            )

    # ---------- contexts 2..n: gather/accumulate segments ----------
    for cs, ce, wlist in seg_bounds:
        with tile.TileContext(nc) as tc, ExitStack() as ctx:
            gp = ctx.enter_context(tc.tile_pool(name="gp", bufs=6))
            cidx = cs
            for w in wlist:
                nb = w // 128
                for gblk in range(nb // 4):
                    stage = gp.tile([128, 4, D], f32)
                    for q in range(4):
                        nc.gpsimd.indirect_dma_start(
                            out=stage[:, q, :],
                            out_offset=None,
                            in_=z_dram[:],
                            in_offset=bass.IndirectOffsetOnAxis(
                                ap=gidx_sb[:, cidx:cidx + 1], axis=0),
                        )
                        cidx += 1
                    blk = gblk * 4
                    nc.vector.tensor_tensor(
                        out=acc[:, blk:blk + 4, :], in0=acc[:, blk:blk + 4, :],
                        in1=stage[:], op=mybir.AluOpType.add)
            assert cidx == ce

    # ---------- final context: scale by dis_dst, +b, relu, store ----------
    with tile.TileContext(nc) as tc, ExitStack() as ctx:
        op = ctx.enter_context(tc.tile_pool(name="op", bufs=3))
        for j in range(NBLK):
            t1 = op.tile([128, D], f32)
            nc.scalar.activation(t1[:], acc[:, j, :],
                                 mybir.ActivationFunctionType.Copy,
                                 scale=dis_loc[:, j:j + 1])
            t2 = op.tile([128, D], f32)
            nc.vector.tensor_tensor(out=t2[:], in0=t1[:], in1=b_full[:],
                                    op=mybir.AluOpType.add)
            t3 = op.tile([128, D], f32)
            nc.vector.tensor_scalar_max(out=t3[:], in0=t2[:], scalar1=0.0)
            nc.sync.dma_start(out=g_out[j * 128:(j + 1) * 128, :], in_=t3[:])


# ---------------------------------------------------------------- runner

def _build_runner(nc, n_cores=NC):
    import jax
    from jax.sharding import Mesh, PartitionSpec
    from jax.experimental.shard_map import shard_map
    import concourse.mybir as mybir
    from concourse import bass2jax

    bass2jax.install_neuronx_cc_hook()
    partition_name = nc.partition_id_tensor.name if nc.partition_id_tensor else None

    in_names, out_names, out_avals, zero_outs = [], [], [], []
    for alloc in nc.m.functions[0].allocations:
        if not isinstance(alloc, mybir.MemoryLocationSet):
            continue
        name = alloc.memorylocations[0].name
        if alloc.kind == "ExternalInput":
            if name != partition_name:
                in_names.append(name)
        elif alloc.kind == "ExternalOutput":
            out_names.append(name)
            shape = tuple(alloc.tensor_shape)
            dtype = mybir.dt.np(alloc.dtype)
            out_avals.append(jax.core.ShapedArray(shape, dtype))
            zero_outs.append(np.zeros(shape, dtype))
    n_params = len(in_names)
    n_outs = len(out_avals)
    all_in_names = list(in_names) + list(out_names)
    if partition_name is not None:
        all_in_names.append(partition_name)

    def _body(*args):
        operands = list(args)
        if partition_name is not None:
            operands.append(bass2jax.partition_id_tensor())
        outs = bass2jax._bass_exec_p.bind(
            *operands,
            out_avals=tuple(out_avals),
            in_names=tuple(all_in_names),
            out_names=tuple(out_names),
            lowering_input_output_aliases=(),
            sim_require_finite=True,
            sim_require_nnan=True,
            nc=nc,
        )
        return tuple(outs)

    devices = jax.devices()[:n_cores]
    mesh = Mesh(np.asarray(devices), ("core",))
    in_specs = (PartitionSpec("core"),) * (n_params + n_outs)
    out_specs = (PartitionSpec("core"),) * n_outs
    donate = tuple(range(n_params, n_params + n_outs))
    fn = jax.jit(
        shard_map(_body, mesh=mesh, in_specs=in_specs, out_specs=out_specs,
                  check_rep=False),
        donate_argnums=donate,
        keep_unused=True,
    )
    sharding = jax.sharding.NamedSharding(mesh, PartitionSpec("core"))

    class R:
        pass

    r = R()
    r.fn = fn
    r.in_names = in_names
    r.out_names = out_names
    r.zero_outs = zero_outs
    r.sharding = sharding
    r.n_cores = n_cores

    def prep_inputs(in_maps):
        import jax as _jax
        return [
            _jax.device_put(
                np.concatenate([np.asarray(m[nm]) for m in in_maps], axis=0),
                sharding)
            for nm in in_names
        ]

    def put_zero_outs():
        import jax as _jax
        return [_jax.device_put(np.concatenate([z] * n_cores, axis=0), sharding)
                for z in zero_outs]

    def run(dev_inputs):
        import jax as _jax
        outs = fn(*dev_inputs, *put_zero_outs())
        outs = [np.asarray(o) for o in outs]
        res = []
        for c in range(n_cores):
            d = {}
            for name, glob in zip(out_names, outs):
                per = glob.shape[0] // n_cores
                d[name] = glob[c * per:(c + 1) * per]
            res.append(d)
        return res

    r.prep_inputs = prep_inputs
    r.put_zero_outs = put_zero_outs
    r.run = run
    return r


# ---------------------------------------------------------------- kernel

def kernel(x, edge_index, W, b):
    x = np.ascontiguousarray(np.asarray(x, dtype=np.float32))
    ei = np.asarray(edge_index)
    W = np.asarray(W, dtype=np.float32)
    b = np.asarray(b, dtype=np.float32)
    row = ei[0].astype(np.int64)
    col = ei[1].astype(np.int64)

    widths, ncols, g_arrs, degloc_arrs, perms, deg_pn = _host_prep(row, col)

    key = (tuple(widths), ncols)
    if key not in _CACHE:
        nc = _build_program(widths, ncols)
        _CACHE[key] = (nc, _build_runner(nc))
    nc, runner = _CACHE[key]

    xT = np.zeros((D, NPADZ), np.float32)
    xT[:, :N_NODES] = x.T
    wt = np.ascontiguousarray(W.T)          # [in, out]
    b2 = b.reshape(1, D)

    in_maps = []
    for k in range(NC):
        in_maps.append({
            "g_xT": xT,
            "g_wt": wt,
            "g_b": b2,
            "g_degpn": deg_pn,
            "g_degloc": degloc_arrs[k],
            "g_gidx": g_arrs[k],
        })
    dev_in = runner.prep_inputs(in_maps)
    res = runner.run(dev_in)

    out = np.empty((N_NODES, D), np.float32)
    for k in range(NC):
        ok = res[k]["g_out"][:SH]
        blk = np.empty((SH, D), np.float32)
        blk[perms[k]] = ok
        out[k * SH:(k + 1) * SH] = blk
    return out
